# revision 1
# baseline (speedup 1.0000x reference)
"""Batched Viterbi decode (BiLSTM-CRF) on 8 Trainium2 NeuronCores.

Data-parallel over batch: each core takes 1024 of the 8192 batch rows.
Per core layout: batch -> (g, p) with b = g*128 + p; 128 partitions x 8
groups, so every per-step vector op covers all 1024 rows at once.

Forward (t = 1..T-1), all on VectorE with wide [128, 1152] ops:
    cand[b,(g,j,i)] = prev[b,(g,i)] + T[i,j]          (stride-0 bcast over j)
    best = segmented reduce_max over i
    D    = cand - best  (bcast over i; D == 0 exactly at the argmax)
    Y    = -2^40 * D + iota_i   (== iota exactly at argmax, huge elsewhere)
    bp_t = segmented reduce_min(Y)   -> first-index argmax, stored bf16
    prev = best + emit_t
Backtrace: pick = sum_j bp_t * onehot; onehot' = (iota == pick).
"""

import os
import sys

import numpy as np

sys.path.insert(0, "/opt/trn_rl_repo")

B_FULL = 8192
T_STEPS = 512
K = 12
N_CORES = 8
B_CORE = B_FULL // N_CORES  # 1024
G = B_CORE // 128  # 8
BIG = float(2.0**40)


def _build_nc(b_core=B_CORE, t_steps=T_STEPS, t_block=32):
    import concourse.bacc as bacc
    import concourse.bass as bass
    import concourse.mybir as mybir
    from concourse import tile
    from concourse.mybir import AluOpType as Op

    g = b_core // 128
    kk = K * K  # 144
    w = g * kk  # wide free size (1152 for full problem)
    w12 = g * K  # 96

    f32 = mybir.dt.float32
    bf16 = mybir.dt.bfloat16
    i32 = mybir.dt.int32

    nc = bacc.Bacc("TRN2", target_bir_lowering=False, debug=False)

    logits = nc.dram_tensor("logits", [b_core, t_steps, K], f32, kind="ExternalInput")
    # transb[p, (g,j,i)] = T[i, j]  (replicated over p and g)
    transb_d = nc.dram_tensor("transb", [128, w], f32, kind="ExternalInput")
    # iota_b[p, (g,j,i)] = i
    iota_b_d = nc.dram_tensor("iota_b", [128, w], f32, kind="ExternalInput")
    # iota12[p, (g,j)] = j
    iota12_d = nc.dram_tensor("iota12", [128, w12], f32, kind="ExternalInput")

    scores_d = nc.dram_tensor("scores", [b_core], f32, kind="ExternalOutput")
    paths_d = nc.dram_tensor("paths", [b_core, t_steps], i32, kind="ExternalOutput")

    n_blocks = t_steps // t_block

    with tile.TileContext(nc) as tc:
        with (
            tc.tile_pool(name="const", bufs=1) as constp,
            tc.tile_pool(name="emit", bufs=2) as emitp,
            tc.tile_pool(name="state", bufs=1) as statep,
            tc.tile_pool(name="scratch", bufs=2) as scrp,
        ):
            transb = constp.tile([128, w], f32, tag="transb")
            iota_b = constp.tile([128, w], f32, tag="iota_b")
            iota12 = constp.tile([128, w12], f32, tag="iota12")
            nc.sync.dma_start(transb[:], transb_d[:])
            nc.sync.dma_start(iota_b[:], iota_b_d[:])
            nc.sync.dma_start(iota12[:], iota12_d[:])

            # persistent state/stores
            bp = statep.tile([128, t_steps, w12], bf16, tag="bp")
            pathsf = statep.tile([128, g, t_steps], f32, tag="pathsf")
            prev = statep.tile([128, g, K], f32, tag="prev")
            h = statep.tile([128, w12], bf16, tag="h")

            def emit_block_dma(tb):
                et = emitp.tile([128, g, t_block, K], f32, tag="emit")
                # dram: logits[g*128+p, tb*t_block + tt, k]
                src = bass.AP(
                    logits,
                    tb * t_block * K,
                    [
                        [t_steps * K, 128],  # p
                        [128 * t_steps * K, g],  # g
                        [K, t_block],  # tt
                        [1, K],  # k
                    ],
                )
                nc.sync.dma_start(et[:], src)
                return et

            for tb in range(n_blocks):
                et = emit_block_dma(tb)
                for tt in range(t_block):
                    t = tb * t_block + tt
                    emit_t = et[:, :, tt, :]  # [128, g, K]
                    if t == 0:
                        nc.vector.tensor_copy(prev[:], emit_t)
                        continue
                    cand = scrp.tile([128, g, K, K], f32, tag="cand")
                    best = scrp.tile([128, g, K], f32, tag="best")
                    dd = scrp.tile([128, g, K, K], f32, tag="dd")
                    yy = scrp.tile([128, g, K, K], f32, tag="yy")
                    # prev broadcast over j: [128, g, K(i)] -> [128, g, K(j), K(i)]
                    prev_b = prev[:].unsqueeze(2).broadcast_to([128, g, K, K])
                    nc.vector.tensor_tensor(
                        cand[:], prev_b, transb[:].rearrange("p (g j i) -> p g j i", g=g, j=K, i=K), Op.add
                    )
                    nc.vector.tensor_reduce(
                        best[:], cand[:], axis=mybir.AxisListType.X, op=Op.max
                    )
                    best_b = best[:].unsqueeze(3).broadcast_to([128, g, K, K])
                    nc.vector.tensor_tensor(dd[:], cand[:], best_b, Op.subtract)
                    nc.vector.scalar_tensor_tensor(
                        yy[:],
                        dd[:],
                        -BIG,
                        iota_b[:].rearrange("p (g j i) -> p g j i", g=g, j=K, i=K),
                        Op.mult,
                        Op.add,
                    )
                    nc.vector.tensor_reduce(
                        bp[:, t, :].rearrange("p (g j) -> p g j", g=g, j=K),
                        yy[:],
                        axis=mybir.AxisListType.X,
                        op=Op.min,
                    )
                    nc.vector.tensor_tensor(prev[:], best[:], emit_t, Op.add)

            # ---- final scores / last tag ----
            scores_sb = statep.tile([128, g], f32, tag="scores_sb")
            dfin = scrp.tile([128, g, K], f32, tag="dfin")
            yfin = scrp.tile([128, g, K], f32, tag="yfin")
            nc.vector.tensor_reduce(
                scores_sb[:], prev[:], axis=mybir.AxisListType.X, op=Op.max
            )
            sc_b = scores_sb[:].unsqueeze(2).broadcast_to([128, g, K])
            nc.vector.tensor_tensor(dfin[:], prev[:], sc_b, Op.subtract)
            nc.vector.scalar_tensor_tensor(
                yfin[:],
                dfin[:],
                -BIG,
                iota12[:].rearrange("p (g j) -> p g j", g=g, j=K),
                Op.mult,
                Op.add,
            )
            # last tag -> pathsf[:, :, T-1]
            nc.vector.tensor_reduce(
                pathsf[:, :, t_steps - 1],
                yfin[:],
                axis=mybir.AxisListType.X,
                op=Op.min,
            )
            nc.sync.dma_start(
                bass.AP(scores_d, 0, [[1, 128], [128, g]]),
                scores_sb[:],
            )

            # ---- backtrace ----
            last_b = (
                pathsf[:, :, t_steps - 1]
                .unsqueeze(2)
                .broadcast_to([128, g, K])
            )
            nc.vector.tensor_tensor(
                h[:].rearrange("p (g j) -> p g j", g=g, j=K),
                iota12[:].rearrange("p (g j) -> p g j", g=g, j=K),
                last_b,
                Op.is_equal,
            )
            for t in range(t_steps - 1, 0, -1):
                mtmp = scrp.tile([128, g, K], f32, tag="mtmp")
                nc.vector.tensor_tensor(
                    mtmp[:],
                    bp[:, t, :].rearrange("p (g j) -> p g j", g=g, j=K),
                    h[:].rearrange("p (g j) -> p g j", g=g, j=K),
                    Op.mult,
                )
                nc.vector.tensor_reduce(
                    pathsf[:, :, t - 1], mtmp[:], axis=mybir.AxisListType.X, op=Op.add
                )
                if t > 1:
                    pick_b = (
                        pathsf[:, :, t - 1].unsqueeze(2).broadcast_to([128, g, K])
                    )
                    nc.vector.tensor_tensor(
                        h[:].rearrange("p (g j) -> p g j", g=g, j=K),
                        iota12[:].rearrange("p (g j) -> p g j", g=g, j=K),
                        pick_b,
                        Op.is_equal,
                    )

            # ---- convert + write paths ----
            paths_i = statep.tile([128, g, t_steps], i32, tag="paths_i")
            nc.vector.tensor_copy(paths_i[:], pathsf[:])
            nc.sync.dma_start(
                bass.AP(
                    paths_d,
                    0,
                    [[t_steps, 128], [128 * t_steps, g], [1, t_steps]],
                ),
                paths_i[:],
            )

    nc.finalize()
    return nc


def _host_consts(transitions, g=G):
    kk = K * K
    transb = np.tile(
        np.transpose(transitions).reshape(1, kk), (128, g)
    ).astype(np.float32)  # (j,i) -> T[i,j], tiled g times per partition
    iota_b = np.tile(np.arange(K, dtype=np.float32), (128, g * K))
    iota12 = np.tile(np.arange(K, dtype=np.float32)[None, :], (128, g)).reshape(
        128, g * K
    )
    # fix iota12: want (g, j) -> j
    iota12 = np.tile(np.arange(K, dtype=np.float32), (128, g))
    return transb, iota_b, iota12


_NC_CACHE = {}


def kernel(logits: np.ndarray, transitions: np.ndarray):
    from concourse import bass_utils

    key = "full"
    if key not in _NC_CACHE:
        _NC_CACHE[key] = _build_nc()
    nc = _NC_CACHE[key]

    transb, iota_b, iota12 = _host_consts(transitions)
    logits = np.ascontiguousarray(np.asarray(logits, dtype=np.float32))

    in_maps = []
    for c in range(N_CORES):
        in_maps.append(
            {
                "logits": logits[c * B_CORE : (c + 1) * B_CORE],
                "transb": transb,
                "iota_b": iota_b,
                "iota12": iota12,
            }
        )
    res = bass_utils.run_bass_kernel_spmd(nc, in_maps, core_ids=list(range(N_CORES)))
    scores = np.concatenate([r["scores"] for r in res.results], axis=0)
    paths = np.concatenate([r["paths"] for r in res.results], axis=0).astype(np.int32)
    return scores.astype(np.float32), paths


# revision 7
# speedup vs baseline: 1298.3427x; 1298.3427x over previous
"""Batched Viterbi decode (BiLSTM-CRF) on 8 Trainium2 NeuronCores.

Data-parallel over batch: each core takes 1024 of the 8192 batch rows.
Per core layout: batch -> (g, p) with b = g*128 + p; 128 partitions x 8
groups, so every per-step vector op covers all 1024 rows at once.

Forward (t = 1..T-1), all on VectorE with wide [128, 1152] ops:
    cand[b,(g,j,i)] = prev[b,(g,i)] + T[i,j]          (stride-0 bcast over j)
    best = segmented reduce_max over i
    D    = cand - best  (bcast over i; D == 0 exactly at the argmax)
    Y    = -2^40 * D + iota_i   (== iota exactly at argmax, huge elsewhere)
    bp_t = segmented reduce_min(Y)   -> first-index argmax, stored bf16
    prev = best + emit_t
Backtrace: pick = sum_j bp_t * onehot; onehot' = (iota == pick).

`repeats` re-runs the whole computation R times inside one NEFF (for
timing by differencing; outputs are identical each repeat).
"""

import sys

import numpy as np

sys.path.insert(0, "/opt/trn_rl_repo")

B_FULL = 8192
T_STEPS = 512
K = 12
N_CORES = 8
B_CORE = B_FULL // N_CORES  # 1024
G = B_CORE // 128  # 8
BIG = float(2.0**40)


def _build_nc(b_core=B_CORE, t_steps=T_STEPS, t_block=32, repeats=1):
    import concourse.bacc as bacc
    import concourse.bass as bass
    import concourse.mybir as mybir
    from concourse import tile
    from concourse.mybir import AluOpType as Op

    g = b_core // 128
    w = g * K * K  # wide free size (1152 for full problem)
    w12 = g * K  # 96

    f32 = mybir.dt.float32
    bf16 = mybir.dt.bfloat16
    i32 = mybir.dt.int32

    nc = bacc.Bacc("TRN2", target_bir_lowering=False, debug=False)

    logits = nc.dram_tensor("logits", [b_core, t_steps, K], f32, kind="ExternalInput")
    # transb[p, (g,j,i)] = T[i, j]  (replicated over p and g)
    transb_d = nc.dram_tensor("transb", [128, w], f32, kind="ExternalInput")
    # iota_b[p, (g,j,i)] = i
    iota_b_d = nc.dram_tensor("iota_b", [128, w], f32, kind="ExternalInput")
    # iota12[p, (g,j)] = j
    iota12_d = nc.dram_tensor("iota12", [128, w12], f32, kind="ExternalInput")

    scores_d = nc.dram_tensor("scores", [b_core], f32, kind="ExternalOutput")
    paths_d = nc.dram_tensor("paths", [b_core, t_steps], i32, kind="ExternalOutput")

    n_blocks = t_steps // t_block

    with tile.TileContext(nc) as tc:
        with (
            tc.tile_pool(name="const", bufs=1) as constp,
            tc.tile_pool(name="emit", bufs=2) as emitp,
            tc.tile_pool(name="state", bufs=1) as statep,
            tc.tile_pool(name="scratch", bufs=2) as scrp,
        ):
            transb = constp.tile([128, w], f32, tag="transb")
            iota_b = constp.tile([128, w], f32, tag="iota_b")
            iota12 = constp.tile([128, w12], f32, tag="iota12")
            nc.sync.dma_start(transb[:], transb_d[:])
            nc.sync.dma_start(iota_b[:], iota_b_d[:])
            nc.sync.dma_start(iota12[:], iota12_d[:])

            transb_v = transb[:].rearrange("p (g j i) -> p g j i", g=g, j=K, i=K)
            iota_b_v = iota_b[:].rearrange("p (g j i) -> p g j i", g=g, j=K, i=K)
            iota12_v = iota12[:].rearrange("p (g j) -> p g j", g=g, j=K)

            def one_pass():
                # state/stores (same tags -> same slots across repeats)
                bp = statep.tile([128, t_steps, w12], bf16, tag="bp")
                pathsf = statep.tile([128, g, t_steps], f32, tag="pathsf")
                prev = statep.tile([128, g, K], f32, tag="prev")
                h = statep.tile([128, w12], bf16, tag="h")

                for tb in range(n_blocks):
                    et = emitp.tile([128, g, t_block, K], f32, tag="emit")
                    src = bass.AP(
                        logits,
                        tb * t_block * K,
                        [
                            [t_steps * K, 128],  # p
                            [128 * t_steps * K, g],  # g
                            [K, t_block],  # tt
                            [1, K],  # k
                        ],
                    )
                    nc.sync.dma_start(et[:], src)
                    for tt in range(t_block):
                        t = tb * t_block + tt
                        emit_t = et[:, :, tt, :]  # [128, g, K]
                        if t == 0:
                            nc.vector.tensor_copy(prev[:], emit_t)
                            continue
                        cand = scrp.tile([128, g, K, K], f32, tag="cand")
                        best = scrp.tile([128, g, K], f32, tag="best")
                        dd = scrp.tile([128, g, K, K], f32, tag="dd")
                        yy = scrp.tile([128, g, K, K], f32, tag="yy")
                        prev_b = prev[:].unsqueeze(2).broadcast_to([128, g, K, K])
                        nc.vector.tensor_tensor(cand[:], prev_b, transb_v, Op.add)
                        nc.vector.tensor_reduce(
                            best[:], cand[:], axis=mybir.AxisListType.X, op=Op.max
                        )
                        best_b = best[:].unsqueeze(3).broadcast_to([128, g, K, K])
                        nc.vector.tensor_tensor(dd[:], cand[:], best_b, Op.subtract)
                        nc.vector.scalar_tensor_tensor(
                            yy[:], dd[:], -BIG, iota_b_v, Op.mult, Op.add
                        )
                        nc.vector.tensor_reduce(
                            bp[:, t, :].rearrange("p (g j) -> p g j", g=g, j=K),
                            yy[:],
                            axis=mybir.AxisListType.X,
                            op=Op.min,
                        )
                        nc.vector.tensor_tensor(prev[:], best[:], emit_t, Op.add)

                # ---- final scores / last tag ----
                scores_sb = statep.tile([128, g], f32, tag="scores_sb")
                dfin = scrp.tile([128, g, K], f32, tag="dfin")
                yfin = scrp.tile([128, g, K], f32, tag="yfin")
                nc.vector.tensor_reduce(
                    scores_sb[:], prev[:], axis=mybir.AxisListType.X, op=Op.max
                )
                sc_b = scores_sb[:].unsqueeze(2).broadcast_to([128, g, K])
                nc.vector.tensor_tensor(dfin[:], prev[:], sc_b, Op.subtract)
                nc.vector.scalar_tensor_tensor(
                    yfin[:], dfin[:], -BIG, iota12_v, Op.mult, Op.add
                )
                nc.vector.tensor_reduce(
                    pathsf[:, :, t_steps - 1],
                    yfin[:],
                    axis=mybir.AxisListType.X,
                    op=Op.min,
                )
                nc.sync.dma_start(
                    bass.AP(scores_d, 0, [[1, 128], [128, g]]),
                    scores_sb[:],
                )

                # ---- backtrace ----
                h_v = h[:].rearrange("p (g j) -> p g j", g=g, j=K)
                last_b = (
                    pathsf[:, :, t_steps - 1].unsqueeze(2).broadcast_to([128, g, K])
                )
                nc.vector.tensor_tensor(h_v, iota12_v, last_b, Op.is_equal)
                for t in range(t_steps - 1, 0, -1):
                    mtmp = scrp.tile([128, g, K], f32, tag="mtmp")
                    nc.vector.tensor_tensor(
                        mtmp[:],
                        bp[:, t, :].rearrange("p (g j) -> p g j", g=g, j=K),
                        h_v,
                        Op.mult,
                    )
                    nc.vector.tensor_reduce(
                        pathsf[:, :, t - 1],
                        mtmp[:],
                        axis=mybir.AxisListType.X,
                        op=Op.add,
                    )
                    if t > 1:
                        pick_b = (
                            pathsf[:, :, t - 1].unsqueeze(2).broadcast_to([128, g, K])
                        )
                        nc.vector.tensor_tensor(h_v, iota12_v, pick_b, Op.is_equal)

                # ---- convert + write paths ----
                paths_i = statep.tile([128, g, t_steps], i32, tag="paths_i")
                nc.vector.tensor_copy(paths_i[:], pathsf[:])
                nc.sync.dma_start(
                    bass.AP(
                        paths_d,
                        0,
                        [[t_steps, 128], [128 * t_steps, g], [1, t_steps]],
                    ),
                    paths_i[:],
                )

            for _rep in range(repeats):
                one_pass()

    nc.finalize()
    return nc


def _host_consts(transitions, g=G):
    kk = K * K
    transb = np.tile(
        np.transpose(np.asarray(transitions, np.float32)).reshape(1, kk), (128, g)
    ).astype(np.float32)
    iota_b = np.tile(np.arange(K, dtype=np.float32), (128, g * K))
    iota12 = np.tile(np.arange(K, dtype=np.float32), (128, g))
    return transb, iota_b, iota12


_NC_CACHE = {}


USE_V2 = True


def kernel(logits: np.ndarray, transitions: np.ndarray):
    from concourse import bass_utils

    logits = np.ascontiguousarray(np.asarray(logits, dtype=np.float32))

    if "full" not in _NC_CACHE:
        _NC_CACHE["full"] = _build_nc()
    nc = _NC_CACHE["full"]
    transb, iota_b, iota12 = _host_consts(transitions)
    in_maps = [
        {
            "logits": logits[c * B_CORE : (c + 1) * B_CORE],
            "transb": transb,
            "iota_b": iota_b,
            "iota12": iota12,
        }
        for c in range(N_CORES)
    ]
    res = bass_utils.run_bass_kernel_spmd(nc, in_maps, core_ids=list(range(N_CORES)))
    scores = np.concatenate([r["scores"] for r in res.results], axis=0)
    paths = np.concatenate([r["paths"] for r in res.results], axis=0).astype(np.int32)
    return scores.astype(np.float32), paths
